# revision 1
# baseline (speedup 1.0000x reference)
# Trainium2 Bass kernel for nn_ClusteringLayer (DEC soft-assignment / Student-t
# codebook posterior):
#   d2[n,k] = ||x_n - c_k||^2 ;  q = 1/(1+d2) row-normalized over k  (alpha=1).
#
# Sharding: data-parallel along N over 8 NeuronCores; clusters replicated.
# Per core: x_shard (16384, 512) f32 -> q_shard (16384, 128) f32.
#
# Per-core plan (orientation: out q[n, k], n on partitions):
#   - x loaded HBM->SBUF with f32->bf16 cast on the SWDGE DMA.
#   - Per 128-row tile: 4 PE transposes produce xT chunks [d,128]; 4 bf16
#     matmuls (lhsT=xT chunk, rhs=-2*cT chunk) + 1 rank-1 matmul
#     (ones x (1+c2)) accumulate psum = 1 + c2[k] - 2 cross.
#   - DVE tensor_scalar: u = max(psum + x2[n], 1.0)  (= 1 + max(d2,0)).
#   - Batched reciprocal / per-tile row-sum / reciprocal; GPSIMD broadcast
#     multiply; f32 DMA out.
import dataclasses

import numpy as np

import concourse.bass as bass
import concourse.mybir as mybir
from concourse import bacc
from concourse.bass import ts
from concourse.masks import make_identity
from concourse.tile import TileContext

N, D, K = 131072, 512, 128
N_CORES = 8
NS = N // N_CORES  # rows per core
P = 128  # partitions / row-tile size
G = 8    # row-tiles per super-tile
F32 = mybir.dt.float32
BF16 = mybir.dt.bfloat16


def _bcast_free(ap: bass.AP, n: int) -> bass.AP:
    """Append a step-0 (broadcast) innermost free dim of size n."""
    return dataclasses.replace(ap, ap=list(ap.ap) + [[0, n]])


def build(ns=NS, g=G, repeat=1, dma_mode="normal", xin_bufs=3, xt_bufs=3,
          ep_bufs=3, qo_bufs=3, ps_t_bufs=2, ps_q_bufs=3):
    n_super = ns // (P * g)
    assert ns == n_super * P * g
    n_dchunk = D // P  # 4

    nc = bacc.Bacc("TRN2", target_bir_lowering=False, debug=False)
    x_dram = nc.dram_tensor("x", [ns, D], F32, kind="ExternalInput")
    c_dram = nc.dram_tensor("clusters", [K, D], F32, kind="ExternalInput")
    q_dram = nc.dram_tensor("q", [ns, K], F32, kind="ExternalOutput")

    with TileContext(nc) as tc:
        with (
            tc.tile_pool(name="const", bufs=1) as const_pool,
            tc.tile_pool(name="xin", bufs=xin_bufs) as xin_pool,
            tc.tile_pool(name="xt", bufs=xt_bufs) as xt_pool,
            tc.tile_pool(name="ep", bufs=ep_bufs) as ep_pool,
            tc.tile_pool(name="qo", bufs=qo_bufs) as qo_pool,
            tc.tile_pool(name="ps_t", bufs=ps_t_bufs, space="PSUM") as ps_t_pool,
            tc.tile_pool(name="ps_q", bufs=ps_q_bufs, space="PSUM") as ps_q_pool,
        ):
            # ---------------- setup (once) ----------------
            if True:
                ps_s_pool = ps_t_pool  # share slots (tag below) to stay <=8 banks
                ident_bf = const_pool.tile([P, P], BF16)
                make_identity(nc, ident_bf)

                c_f32 = const_pool.tile([K, D], F32)
                nc.sync.dma_start(c_f32[:], c_dram[:, :])
                c_bf = const_pool.tile([K, D], BF16)
                nc.vector.tensor_copy(c_bf[:], c_f32[:])

                # c2[k] = sum_d c_bf[k,d]^2 (fp32 accum), then 1 + c2 as bf16
                csq = const_pool.tile([K, D], F32)
                c2 = const_pool.tile([K, 1], F32)
                nc.scalar.activation(
                    csq[:], c_bf[:], mybir.ActivationFunctionType.Square,
                    accum_out=c2[:],
                )
                c2p1_bf = const_pool.tile([K, 1], BF16)
                nc.vector.tensor_scalar_add(c2p1_bf[:], c2[:], 1.0)

                # transpose (1+c2) -> row [1, K] bf16
                ps_row = ps_s_pool.tile([1, K], BF16, tag="ps_xt")
                nc.tensor.transpose(ps_row[:], c2p1_bf[:], ident_bf[:])
                c2p1_row = const_pool.tile([1, K], BF16)
                nc.vector.tensor_copy(c2p1_row[:], ps_row[:])

                ones_row = const_pool.tile([1, K], BF16)
                nc.vector.memset(ones_row[:], 1.0)

                # cTm2[p, c, k] = -2 * clusters_bf[k, c*128+p]
                cTm2 = const_pool.tile([P, n_dchunk, K], BF16)
                for c in range(n_dchunk):
                    ps_c = ps_s_pool.tile([P, P], BF16, tag="ps_xt")
                    nc.tensor.transpose(ps_c[:], c_bf[:, ts(c, P)], ident_bf[:])
                    nc.vector.tensor_scalar_mul(cTm2[:, c, :], ps_c[:], -2.0)

            # ---------------- main loop ----------------
            # x loads are software-pipelined: st+1's SWDGE cast-load is issued
            # at the START of st's body so it lands ahead of the gpsimd
            # multiplies in the Pool queue and overlaps st's compute.
            def issue_load(sti):
                n0 = (sti % n_super) * P * g
                x_view = x_dram[n0:n0 + P * g, :].rearrange(
                    "(gg p) d -> p gg d", p=P)
                t = xin_pool.tile([P, g, D], BF16, name="x_nat", tag="x_nat")
                nc.gpsimd.dma_start(t[:], x_view)
                return t

            n_total = n_super * repeat
            if dma_mode == "once":
                x_once = issue_load(0)
            else:
                pending = issue_load(0)
            for sti in range(n_total):
                st = sti % n_super
                n0 = st * P * g
                if dma_mode == "once":
                    x_nat = x_once
                else:
                    x_nat = pending
                    if sti + 1 < n_total:
                        pending = issue_load(sti + 1)

                x2s = ep_pool.tile([P, g], F32, tag="x2s")
                u = ep_pool.tile([P, g, K], F32, tag="u")
                psum_q = ps_q_pool.tile([P, g, K], F32)

                for gp in range(g // 2):  # process row-tiles in pairs
                    # one PSUM bank holds both tiles' transposed chunks
                    ps_xt = ps_t_pool.tile([P, 2 * n_dchunk, P], BF16)
                    for gg in (2 * gp, 2 * gp + 1):
                        half = n_dchunk * (gg & 1)
                        # x2 via ACT: square + fp32 accum (scratch discarded)
                        sq_scr = xt_pool.tile([P, D], BF16, tag="sq")
                        nc.scalar.activation(
                            sq_scr[:], x_nat[:, gg, :],
                            mybir.ActivationFunctionType.Square,
                            accum_out=x2s[:, gg:gg + 1],
                        )
                        for c in range(n_dchunk):
                            nc.tensor.transpose(
                                ps_xt[:, half + c, :], x_nat[:, gg, ts(c, P)],
                                ident_bf[:])
                    xt2 = xt_pool.tile([P, 2 * n_dchunk, P], BF16, tag="xt")
                    nc.vector.tensor_copy(xt2[:], ps_xt[:])

                    for gg in (2 * gp, 2 * gp + 1):
                        half = n_dchunk * (gg & 1)
                        # 4 bf16 matmuls + rank-1 (1+c2) row
                        for c in range(n_dchunk):
                            nc.tensor.matmul(
                                psum_q[:, gg, :], lhsT=xt2[:, half + c, :],
                                rhs=cTm2[:, c, :], start=(c == 0), stop=False)
                        nc.tensor.matmul(
                            psum_q[:, gg, :], lhsT=ones_row[:],
                            rhs=c2p1_row[:], start=False, stop=True)

                # u = psum + x2[n]  broadcast along k, one op per super-tile
                # (clamp at 1.0 skipped: d2 >= -1e-4 numerically; ref's
                # max(d2,0) differs by <=1e-5 rel)
                nc.vector.tensor_tensor(
                    out=u[:], in0=psum_q[:],
                    in1=_bcast_free(x2s[:], K),
                    op=mybir.AluOpType.add,
                )

                # epilogue (batched over g)
                qun = ep_pool.tile([P, g, K], F32, tag="qun")
                nc.vector.reciprocal(qun[:], u[:])
                s8 = ep_pool.tile([P, g], F32, tag="s8")
                nc.vector.tensor_reduce(
                    s8[:], qun[:], axis=mybir.AxisListType.X,
                    op=mybir.AluOpType.add)
                r8 = ep_pool.tile([P, g], F32, tag="r8")
                nc.vector.reciprocal(r8[:], s8[:])

                qout = qo_pool.tile([P, g, K], F32)
                nc.gpsimd.tensor_tensor(
                    out=qout[:], in0=qun[:], in1=_bcast_free(r8[:], K),
                    op=mybir.AluOpType.mult)

                q_view = q_dram[n0:n0 + P * g, :].rearrange(
                    "(gg p) k -> p gg k", p=P)
                nc.sync.dma_start(q_view, qout[:])

    nc.compile()
    return nc


_CACHE = {}


def _get_nc():
    if "nc" not in _CACHE:
        _CACHE["nc"] = build()
    return _CACHE["nc"]


def kernel(x: np.ndarray, clusters: np.ndarray) -> np.ndarray:
    from concourse.bass_utils import run_bass_kernel_spmd

    x = np.ascontiguousarray(x, dtype=np.float32)
    clusters = np.ascontiguousarray(clusters, dtype=np.float32)
    nc = _get_nc()
    in_maps = [
        {"x": x[i * NS:(i + 1) * NS], "clusters": clusters}
        for i in range(N_CORES)
    ]
    res = run_bass_kernel_spmd(nc, in_maps, core_ids=list(range(N_CORES)))
    return np.concatenate([r["q"] for r in res.results], axis=0)



# revision 3
# speedup vs baseline: 2.2959x; 2.2959x over previous
# Trainium2 Bass kernel for nn_ClusteringLayer (DEC soft-assignment):
#   d2[n,k] = ||x_n - c_k||^2 ;  q = 1/(1+d2) row-normalized over k (alpha=1).
# Data-parallel along N over 8 NeuronCores; clusters replicated.
# Per core: x_shard (16384, 512) f32 -> q_shard (16384, 128) f32.
#  - x2 folded into the PE accumulation: x2s_wide [P, 2g] interleaves
#    (x2[n], 1.0) pairs; one small PE transpose per super-tile yields
#    x2one [2g, P] bf16, and a rank-2 matmul per row-tile adds
#    x2[n]*1[k] + 1[n]*(1+c2)[k] directly into psum. Kills the DVE
#    broadcast-add (and the u tile).
#  - squares split: even gg on ACT (Square+accum), odd gg on DVE
#    (tensor_tensor_reduce mult+add) to balance engines.
#  - reciprocal_approx_fast for qun (from PSUM) and r8.
#  - (p gg) DMA layout for 16KB-contiguous input descriptors.
import dataclasses

import numpy as np

import concourse.bass as bass
import concourse.mybir as mybir
from concourse import bacc
from concourse.bass import ts
from concourse.masks import make_identity
from concourse.tile import TileContext

N, D, K = 131072, 512, 128
N_CORES = 8
NS = N // N_CORES  # rows per core
P = 128  # partitions / row-tile size
G = 8    # row-tiles per super-tile
F32 = mybir.dt.float32
BF16 = mybir.dt.bfloat16


def _bcast_free(ap: bass.AP, n: int) -> bass.AP:
    """Append a step-0 (broadcast) innermost free dim of size n."""
    return dataclasses.replace(ap, ap=list(ap.ap) + [[0, n]])


def build(ns=NS, g=G, repeat=1, dma_mode="normal", act_sq=8, xin_bufs=3,
          xt_bufs=3, ep_bufs=3, qo_bufs=3, ps_t_bufs=2, ps_q_bufs=3,
          token=False):
    n_super = ns // (P * g)
    assert ns == n_super * P * g
    n_dchunk = D // P  # 4

    nc = bacc.Bacc("TRN2", target_bir_lowering=False, debug=False)
    x_dram = nc.dram_tensor("x", [ns, D], F32, kind="ExternalInput")
    c_dram = nc.dram_tensor("clusters", [K, D], F32, kind="ExternalInput")
    q_dram = nc.dram_tensor("q", [ns, K], F32, kind="ExternalOutput")
    if token:
        tok_i = nc.dram_tensor("tok", [1, 1], F32, kind="ExternalInput")
        tok_o = nc.dram_tensor("tok_out", [1, 1], F32, kind="ExternalOutput")

    with TileContext(nc) as tc:
        with (
            tc.tile_pool(name="const", bufs=1) as const_pool,
            tc.tile_pool(name="xin", bufs=xin_bufs) as xin_pool,
            tc.tile_pool(name="xt", bufs=xt_bufs) as xt_pool,
            tc.tile_pool(name="ep", bufs=ep_bufs) as ep_pool,
            tc.tile_pool(name="qo", bufs=qo_bufs) as qo_pool,
            tc.tile_pool(name="ps_t", bufs=ps_t_bufs, space="PSUM") as ps_t_pool,
            tc.tile_pool(name="ps_q", bufs=ps_q_bufs, space="PSUM") as ps_q_pool,
        ):
            # ---------------- setup (once) ----------------
            if True:
                ps_s_pool = ps_t_pool  # share slots (tag below) to stay <=8 banks
                ident_bf = const_pool.tile([P, P], BF16)
                make_identity(nc, ident_bf)

                c_f32 = const_pool.tile([K, D], F32)
                nc.sync.dma_start(c_f32[:], c_dram[:, :])
                c_bf = const_pool.tile([K, D], BF16)
                nc.vector.tensor_copy(c_bf[:], c_f32[:])

                # c2[k] = sum_d c_bf[k,d]^2 (fp32 accum), then 1 + c2 as bf16
                csq = const_pool.tile([K, D], F32)
                c2 = const_pool.tile([K, 1], F32)
                nc.scalar.activation(
                    csq[:], c_bf[:], mybir.ActivationFunctionType.Square,
                    accum_out=c2[:],
                )
                c2p1_bf = const_pool.tile([K, 1], BF16)
                nc.vector.tensor_scalar_add(c2p1_bf[:], c2[:], 1.0)

                # Per-gg selector rhs for the rank-16 MM: row 2gg = ones,
                # row 2gg+1 = 1+c2, all other rows zero. Contracting the
                # full [2g, P] x2one tile against sel[gg] adds
                # x2[n]*1[k] + 1[n]*(1+c2)[k] into psum for row-tile gg.
                # (Built transposed [K, 2g] first — partition-base-0 ops
                # only — then PE-transposed to [2g, K].)
                onec2_sel = []
                for gg in range(g):
                    selT = const_pool.tile([K, 2 * g], BF16, name=f"selT{gg}")
                    nc.vector.memset(selT[:], 0.0)
                    nc.vector.memset(selT[:, 2 * gg:2 * gg + 1], 1.0)
                    nc.vector.tensor_copy(selT[:, 2 * gg + 1:2 * gg + 2],
                                          c2p1_bf[:])
                    ps_sel = ps_s_pool.tile([2 * g, K], BF16, tag="ps_xt")
                    nc.tensor.transpose(ps_sel[:], selT[:], ident_bf[:])
                    sel = const_pool.tile([2 * g, K], BF16, name=f"sel{gg}")
                    nc.vector.tensor_copy(sel[:], ps_sel[:])
                    onec2_sel.append(sel)

                # cTm2[p, c, k] = -2 * clusters_bf[k, c*128+p]
                cTm2 = const_pool.tile([P, n_dchunk, K], BF16)
                for c in range(n_dchunk):
                    ps_c = ps_s_pool.tile([P, P], BF16, tag="ps_xt")
                    nc.tensor.transpose(ps_c[:], c_bf[:, ts(c, P)], ident_bf[:])
                    nc.vector.tensor_scalar_mul(cTm2[:, c, :], ps_c[:], -2.0)

            # ---------------- main loop ----------------
            def issue_load(sti):
                n0 = (sti % n_super) * P * g
                x_view = x_dram[n0:n0 + P * g, :].rearrange(
                    "(p gg) d -> p gg d", p=P)
                t = xin_pool.tile([P, g, D], BF16, name="x_nat", tag="x_nat")
                nc.gpsimd.dma_start(t[:], x_view)
                return t

            n_total = n_super * repeat
            if dma_mode == "once":
                x_once = issue_load(0)
            else:
                pending = issue_load(0)
            for sti in range(n_total):
                st = sti % n_super
                n0 = st * P * g
                if dma_mode == "once":
                    x_nat = x_once
                else:
                    x_nat = pending
                    if sti + 1 < n_total:
                        pending = issue_load(sti + 1)

                # x2s_wide[:, 2gg] = x2 for row-tile gg, [:, 2gg+1] = 1.0
                x2s_wide = ep_pool.tile([P, 2 * g], F32, tag="x2w")
                nc.gpsimd.memset(x2s_wide[:], 1.0)
                psum_q = ps_q_pool.tile([P, g, K], F32)

                # phase 1: squares (ACT/DVE) + PE transposes, per pair
                xt2s = []
                for gp in range(g // 2):
                    ps_xt = ps_t_pool.tile([P, 2 * n_dchunk, P], BF16)
                    for gg in (2 * gp, 2 * gp + 1):
                        half = n_dchunk * (gg & 1)
                        sq_scr = xt_pool.tile([P, D], BF16, tag="sq")
                        if gg < act_sq:
                            # x2 via ACT: square + fp32 accum
                            nc.scalar.activation(
                                sq_scr[:], x_nat[:, gg, :],
                                mybir.ActivationFunctionType.Square,
                                accum_out=x2s_wide[:, 2 * gg:2 * gg + 1],
                            )
                        else:
                            # x2 via DVE: fused mult + reduce
                            nc.vector.tensor_tensor_reduce(
                                out=sq_scr[:],
                                in0=x_nat[:, gg, :], in1=x_nat[:, gg, :],
                                scale=1.0, scalar=0.0,
                                op0=mybir.AluOpType.mult,
                                op1=mybir.AluOpType.add,
                                accum_out=x2s_wide[:, 2 * gg:2 * gg + 1],
                            )
                        for c in range(n_dchunk):
                            nc.tensor.transpose(
                                ps_xt[:, half + c, :], x_nat[:, gg, ts(c, P)],
                                ident_bf[:])
                    xt2 = xt_pool.tile([P, 2 * n_dchunk, P], BF16,
                                       tag=f"xt{gp}")
                    nc.vector.tensor_copy(xt2[:], ps_xt[:])
                    xt2s.append(xt2)

                # x2one[2gg, :] = x2 row, x2one[2gg+1, :] = 1.0 row (bf16)
                x2w_bf = ep_pool.tile([P, 2 * g], BF16, tag="x2wb")
                nc.vector.tensor_copy(x2w_bf[:], x2s_wide[:])
                ps_w = ps_t_pool.tile([2 * g, P], BF16, tag="ps_xt")
                nc.tensor.transpose(ps_w[:], x2w_bf[:], ident_bf[:])
                x2one = ep_pool.tile([2 * g, P], BF16, tag="x2one")
                nc.vector.tensor_copy(x2one[:], ps_w[:])

                # phase 2: MM streams, one accumulation group per gg:
                # 4 cross-chunk MMs then the rank-16 selector MM
                # (psum += x2[n]*1[k] + 1[n]*(1+c2)[k]) closing the group.
                for gg in range(g):
                    xt2 = xt2s[gg // 2]
                    half = n_dchunk * (gg & 1)
                    for c in range(n_dchunk):
                        nc.tensor.matmul(
                            psum_q[:, gg, :], lhsT=xt2[:, half + c, :],
                            rhs=cTm2[:, c, :], start=(c == 0), stop=False)
                    nc.tensor.matmul(
                        psum_q[:, gg, :], lhsT=x2one[:],
                        rhs=onec2_sel[gg][:], start=False, stop=True)

                # epilogue (batched over g): qun = 1/psum straight from PSUM
                qun = ep_pool.tile([P, g, K], F32, tag="qun")
                nc.vector.reciprocal_approx_fast(qun[:], psum_q[:])
                s8 = ep_pool.tile([P, g], F32, tag="s8")
                nc.vector.tensor_reduce(
                    s8[:], qun[:], axis=mybir.AxisListType.X,
                    op=mybir.AluOpType.add)
                r8 = ep_pool.tile([P, g], F32, tag="r8")
                nc.vector.reciprocal_approx_fast(r8[:], s8[:])

                qout = qo_pool.tile([P, g, K], F32)
                nc.gpsimd.tensor_tensor(
                    out=qout[:], in0=qun[:], in1=_bcast_free(r8[:], K),
                    op=mybir.AluOpType.mult)

                q_view = q_dram[n0:n0 + P * g, :].rearrange(
                    "(p gg) k -> p gg k", p=P)
                nc.sync.dma_start(q_view, qout[:])

        if token:
            with tc.tile_pool(name="tokp", bufs=1) as tok_pool:
                tok_sb = tok_pool.tile([1, 1], F32)
                nc.sync.dma_start(tok_sb[:], tok_i[:, :])
                nc.sync.dma_start(tok_o[:, :], tok_sb[:])

    nc.compile()
    return nc


_CACHE = {}


def _get_nc():
    if "nc" not in _CACHE:
        _CACHE["nc"] = build()
    return _CACHE["nc"]


def kernel(x: np.ndarray, clusters: np.ndarray) -> np.ndarray:
    from concourse.bass_utils import run_bass_kernel_spmd

    x = np.ascontiguousarray(x, dtype=np.float32)
    clusters = np.ascontiguousarray(clusters, dtype=np.float32)
    nc = _get_nc()
    in_maps = [
        {"x": x[i * NS:(i + 1) * NS], "clusters": clusters}
        for i in range(N_CORES)
    ]
    res = run_bass_kernel_spmd(nc, in_maps, core_ids=list(range(N_CORES)))
    return np.concatenate([r["q"] for r in res.results], axis=0)
